# revision 13
# baseline (speedup 1.0000x reference)
"""Binarized MLP forward (BinaryConnect, training-mode BatchNorm) on 8 TRN2 cores.

Data-parallel over the batch (16384 -> 8 x 2048), weights replicated and
host-binarized to +-1 f32r (the BIR verifier requires matching f32-family
dtypes on both matmul operands).  Activations stay TRANSPOSED on device
([features, batch]) in f32r.

Per layer the matmul loop is k-outer / b-inner so one stationary weight tile
serves 4 matmuls (LDWEIGHTS amortized).  Per-feature batch statistics are
all-reduced in two groups (j0..4 after j4; j5..7 at the layer end).  The
~12-17us ncfw mesh round-trip of the tail group is hidden by a "runway": the
next layer's first j-columns run their k0..4 contraction (gated only on the
early group) before any k5..7 matmul is enqueued.  Engine-FIFO discipline:
 - GpSimd queue carries ONLY collective cin-DMAs + triggers (a trigger
   retires when its mesh completes, blocking everything behind it),
 - collective results are read back on the Sync queue,
 - all BN-parameter math (rsqrt via Ln/Exp, scale/shift) runs on ScalarE
   with per-partition scale/bias operands, so the VectorE FIFO only ever
   holds local-dependency work (PSUM drains + bn_stats).
"""
import numpy as np

import concourse.bass as bass
import concourse.bacc as bacc
import concourse.tile as tile
from concourse.tile_rust import add_dep_helper
import concourse.mybir as mybir
from concourse.bass_utils import run_bass_kernel_spmd

N_CORES = 8
B_TOT = 16384
BPC = B_TOT // N_CORES  # 2048 batch rows per core
NB = BPC // 512  # 4 free-dim tiles of 512
D_IN, H, D_OUT = 784, 1024, 10
D_IN_PAD = 896  # pad 784 -> 7 full k-tiles of 128
KT1 = D_IN_PAD // 128
NJ = H // 128  # 8 feature tiles per hidden layer
BN_EPS = 1e-5
KA = 5  # k-tiles gated only on the previous layer's early stats group
RUNWAY_SPLIT = (1,)  # j-columns that spill a k0..KA-1 partial

f32 = mybir.dt.float32
f32r = mybir.dt.float32r
AF = mybir.ActivationFunctionType
ALU = mybir.AluOpType

# t_vec scratch layout (free-dim float offsets), indexed + j in 0..7
V_S = 0     # BN scale per feature tile
V_T = 8     # BN shift
V_M = 16    # mean
V_E2 = 24   # E[x^2]
V_VU = 32   # var + eps
V_LN = 40   # ln(var+eps)
V_R = 48    # rsqrt
V_MT = 56   # scratch (mean*s etc.)
V_TMP = 64
V_WRM = 71  # act-table warmup scratch


def build(nc):
    xT = nc.dram_tensor("xT", [KT1, NB, 128, 512], f32r, kind="ExternalInput")
    w1s = nc.dram_tensor("w1s", [128, KT1, H], f32r, kind="ExternalInput")
    w2s = nc.dram_tensor("w2s", [128, NJ, H], f32r, kind="ExternalInput")
    w3s = nc.dram_tensor("w3s", [128, NJ, H], f32r, kind="ExternalInput")
    w4s = nc.dram_tensor("w4s", [128, NJ, D_OUT], f32r, kind="ExternalInput")
    gbp = nc.dram_tensor("gbp", [128, 6, 8], f32, kind="ExternalInput")
    outT = nc.dram_tensor("outT", [D_OUT, BPC], f32, kind="ExternalOutput")

    rg = [list(range(N_CORES))]

    with tile.TileContext(nc) as tc:
        with (
            tc.tile_pool(name="hp", bufs=2) as hpool,
            tc.tile_pool(name="wp", bufs=2) as wpool,
            tc.tile_pool(name="w4p", bufs=1) as w4pool,
            tc.tile_pool(name="pt", bufs=4) as partpool,
            tc.tile_pool(name="outp", bufs=2) as outpool,
            tc.tile_pool(name="msc", bufs=1) as mpool,
            tc.tile_pool(name="ps", bufs=8, space="PSUM") as pspool,
            tc.tile_pool(name="dram", bufs=1, space="DRAM") as dpool,
        ):
            t_stats = mpool.tile([128, 192], f32, name="t_stats")
            t_part = mpool.tile([128, 16], f32, name="t_part")
            t_gst = mpool.tile([128, 16], f32, name="t_gst")
            t_vec = mpool.tile([128, 72], f32, name="t_vec")
            t_gb = mpool.tile([128, 48], f32, name="t_gb")

            # --- warmup: ncfw first-call cost + ACT spline tables used later
            with nc.named_scope("warmup"):
                win = dpool.tile([128, 2], f32, name="warm_in")
                wout = dpool.tile([128, 2], f32, name="warm_out", addr_space="Shared")
                nc.gpsimd.collective_compute(
                    "AllReduce", ALU.add, replica_groups=rg,
                    ins=[win[:].opt()], outs=[wout[:].opt()],
                )
                wsl = t_vec[:, V_WRM: V_WRM + 1]
                nc.vector.memset(wsl, 1.0)
                for fn in (AF.Ln, AF.Exp, AF.Relu, AF.Sigmoid):
                    nc.scalar.activation(wsl, wsl, fn)

            # --- SBUF tiles
            xT_t = hpool.tile([128, KT1, NB, 512], f32r, name="xT_t", tag="h")
            h1 = hpool.tile([128, NJ, NB, 512], f32r, name="h1", tag="h")
            h2 = hpool.tile([128, NJ, NB, 512], f32r, name="h2", tag="h")
            h3 = hpool.tile([128, NJ, NB, 512], f32r, name="h3", tag="h")
            W1t = wpool.tile([128, KT1, H], f32r, name="W1t", tag="w")
            W2t = wpool.tile([128, NJ, H], f32r, name="W2t", tag="w")
            W3t = wpool.tile([128, NJ, H], f32r, name="W3t", tag="w")
            W4t = w4pool.tile([128, NJ, D_OUT], f32r, name="W4t")

            # --- input DMAs in first-consumer order (sync queue is FIFO);
            # w3 is deferred into L2's emission (see hidden_layer hook).
            with nc.named_scope("in_dma"):
                for k in range(KT1):
                    nc.sync.dma_start(W1t[:, k], w1s[:, k])
                    nc.sync.dma_start(xT_t[:, k, 0], xT[k, 0])
                for k in range(KT1):
                    nc.sync.dma_start(
                        xT_t[:, k, 1:NB],
                        xT[k, 1:NB].rearrange("b p c -> p b c"),
                    )
                nc.sync.dma_start(t_gb[:], gbp[:].rearrange("p a b -> p (a b)"))
                for k in range(NJ):
                    nc.sync.dma_start(W2t[:, k], w2s[:, k])
                nc.sync.dma_start(
                    W4t[:].rearrange("p a b -> p (a b)"),
                    w4s[:].rearrange("p a b -> p (a b)"),
                )

            def load_w3():
                with nc.named_scope("w3_dma"):
                    for k in range(NJ):
                        nc.sync.dma_start(W3t[:, k], w3s[:, k])

            # ---------- helpers ----------
            def drain_stats(li, j, accs, out_h, bs, stats_first=False, act_bs=()):
                def stats(b, src):
                    so = j * 24 + b * 6
                    nc.vector.bn_stats(t_stats[:, so: so + 6], src)

                if stats_first:
                    for b in bs:
                        stats(b, accs[b][:])
                for b in bs:
                    if b in act_bs:
                        nc.scalar.copy(out_h[:, j, b], accs[b][:])
                    else:
                        nc.vector.tensor_copy(out_h[:, j, b], accs[b][:])
                if not stats_first:
                    for b in bs:
                        stats(b, out_h[:, j, b].bitcast(f32))

            def aggr(li, j):
                po = j * 2
                nc.vector.bn_aggr(
                    t_part[:, po: po + 2], t_stats[:, j * 24: j * 24 + 24]
                )
                nc.vector.tensor_tensor(
                    t_vec[:, V_TMP + j: V_TMP + j + 1],
                    t_part[:, po: po + 1],
                    t_part[:, po: po + 1],
                    op=ALU.mult,
                )
                nc.vector.tensor_tensor(
                    t_part[:, po + 1: po + 2],
                    t_vec[:, V_TMP + j: V_TMP + j + 1],
                    t_part[:, po + 1: po + 2],
                    op=ALU.add,
                )

            last_gp = [None]  # last gpsimd st-op: cin DMAs are pinned after it

            def collective_group(li, j_lo, j_hi, gtag):
                """gpsimd: stage partials to DRAM + trigger; readback on sync."""
                n = (j_hi - j_lo) * 2
                with nc.named_scope(f"L{li}_ar{gtag}"):
                    cin = dpool.tile([128, n], f32, name=f"cin{li}{gtag}")
                    cout = dpool.tile(
                        [128, n], f32, name=f"cout{li}{gtag}", addr_space="Shared"
                    )
                    ci = nc.gpsimd.dma_start(cin[:], t_part[:, j_lo * 2: j_hi * 2])
                    if last_gp[0] is not None:
                        add_dep_helper(
                            ci.ins, last_gp[0].ins, False,
                            "st gpsimd math precedes next collective stage-in",
                        )
                        last_gp[0] = None
                    nc.gpsimd.collective_compute(
                        "AllReduce", ALU.add, replica_groups=rg,
                        ins=[cin[:].opt()], outs=[cout[:].opt()],
                    )
                    nc.sync.dma_start(t_gst[:, j_lo * 2: j_hi * 2], cout[:])

            def st_apply_group(li, j_lo, j_hi, gtag, out_h):
                """BN param math: batched GpSimd TT/TS + ScalarE Ln/Exp rsqrt,
                then one Relu apply per feature tile on ScalarE.
                s = g*rsqrt(var+eps), t = b - mean*s, h = Relu(s*z + t)."""
                w = j_hi - j_lo
                with nc.named_scope(f"L{li}_st{gtag}"):
                    gview = t_gst[:, j_lo * 2: j_hi * 2].rearrange(
                        "p (j c) -> p j c", c=2
                    )
                    mm = t_vec[:, V_M + j_lo: V_M + j_hi]
                    e2 = t_vec[:, V_E2 + j_lo: V_E2 + j_hi]
                    vu = t_vec[:, V_VU + j_lo: V_VU + j_hi]
                    lnv = t_vec[:, V_LN + j_lo: V_LN + j_hi]
                    rr = t_vec[:, V_R + j_lo: V_R + j_hi]
                    tp2 = t_vec[:, V_MT + j_lo: V_MT + j_hi]
                    sv = t_vec[:, V_S + j_lo: V_S + j_hi]
                    tv = t_vec[:, V_T + j_lo: V_T + j_hi]
                    g_sl = t_gb[:, (li - 1) * 16 + j_lo: (li - 1) * 16 + j_hi]
                    b_sl = t_gb[:, (li - 1) * 16 + 8 + j_lo: (li - 1) * 16 + 8 + j_hi]
                    nc.gpsimd.tensor_scalar(mm, gview[:, :, 0], 1.0 / N_CORES, None, op0=ALU.mult)
                    nc.gpsimd.tensor_scalar(e2, gview[:, :, 1], 1.0 / N_CORES, None, op0=ALU.mult)
                    nc.gpsimd.tensor_tensor(tp2, mm, mm, op=ALU.mult)
                    nc.gpsimd.tensor_tensor(vu, e2, tp2, op=ALU.subtract)
                    nc.gpsimd.tensor_scalar(vu, vu, BN_EPS, None, op0=ALU.add)
                    nc.scalar.activation(lnv, vu, AF.Ln)
                    nc.scalar.activation(rr, lnv, AF.Exp, scale=-0.5)
                    nc.gpsimd.tensor_tensor(sv, g_sl, rr, op=ALU.mult)
                    nc.gpsimd.tensor_tensor(tp2, mm, sv, op=ALU.mult)
                    last_gp[0] = nc.gpsimd.tensor_tensor(tv, b_sl, tp2, op=ALU.subtract)
                with nc.named_scope(f"L{li}_apply{gtag}"):
                    for j in range(j_lo, j_hi):
                        hv = out_h[:, j].rearrange("p b c -> p (b c)")
                        nc.scalar.activation(
                            hv, hv.bitcast(f32), AF.Relu,
                            bias=t_vec[:, V_T + j: V_T + j + 1],
                            scale=t_vec[:, V_S + j: V_S + j + 1],
                        )

            def mm_block(accs, Wt, jcols, rhs, ks, nkt, bs=range(NB), start0=None):
                for k in ks:
                    st = (k == 0) if start0 is None else (k == ks[0] and start0)
                    for b in bs:
                        nc.tensor.matmul(
                            accs[b][:],
                            Wt[:, k, jcols],
                            rhs[:, k, b],
                            start=st,
                            stop=(k == nkt - 1),
                        )

            # ---------- layer 1: phase A (b0) then phase B (b1..3) ----------
            with nc.named_scope("L1_mmA"):
                accA = [
                    pspool.tile([128, 512], f32, name=f"psA_j{j}", tag="ps")
                    for j in range(NJ)
                ]
                for k in range(KT1):
                    for j in range(NJ):
                        nc.tensor.matmul(
                            accA[j][:],
                            W1t[:, k, j * 128: (j + 1) * 128],
                            xT_t[:, k, 0],
                            start=(k == 0),
                            stop=(k == KT1 - 1),
                        )
                for j in range(NJ):
                    drain_stats(1, j, {0: accA[j]}, h1, [0])
            with nc.named_scope("L1_mmB"):
                for j in range(NJ):
                    accB = {
                        b: pspool.tile([128, 512], f32, name=f"psB_j{j}_b{b}", tag="ps")
                        for b in range(1, NB)
                    }
                    jc = slice(j * 128, (j + 1) * 128)
                    mm_block(accB, W1t, jc, xT_t, list(range(KT1)), KT1,
                             bs=range(1, NB))
                    drain_stats(1, j, accB, h1, list(range(1, NB)),
                                stats_first=(j >= 5))
                    aggr(1, j)
                    if j == 4:
                        collective_group(1, 0, 5, "a")
            st_apply_group(1, 0, 5, "a", h1)
            collective_group(1, 5, NJ, "b")
            st_apply_group(1, 5, NJ, "b", h1)

            # ---------- hidden layers 2,3 with boundary runway ----------
            def hidden_layer(li, Wt, rhs, out_h, after_ara=None):
                accs0 = {
                    b: pspool.tile([128, 512], f32, name=f"ps_l{li}j0_b{b}", tag="ps")
                    for b in range(NB)
                }
                with nc.named_scope(f"L{li}_run"):
                    mm_block(accs0, Wt, slice(0, 128), rhs, list(range(KA)), NJ)
                    parts = {}
                    for js in RUNWAY_SPLIT:
                        pacc = {
                            b: pspool.tile(
                                [128, 512], f32, name=f"ps_l{li}p{js}_b{b}", tag="ps"
                            )
                            for b in range(NB)
                        }
                        jc = slice(js * 128, (js + 1) * 128)
                        for k in range(KA):
                            for b in range(NB):
                                nc.tensor.matmul(
                                    pacc[b][:], Wt[:, k, jc], rhs[:, k, b],
                                    start=(k == 0), stop=(k == KA - 1),
                                )
                        parts[js] = {}
                        for b in range(NB):
                            pt = partpool.tile(
                                [128, 512], f32, name=f"part{li}_{js}_{b}", tag="part"
                            )
                            nc.vector.tensor_copy(pt[:], pacc[b][:])
                            parts[js][b] = pt
                with nc.named_scope(f"L{li}_mm"):
                    mm_block(accs0, Wt, slice(0, 128), rhs, list(range(KA, NJ)), NJ,
                             start0=False)
                    drain_stats(li, 0, accs0, out_h, list(range(NB)))
                    aggr(li, 0)
                    for js in RUNWAY_SPLIT:
                        pacc2 = {
                            b: pspool.tile(
                                [128, 512], f32, name=f"ps_l{li}q{js}_b{b}", tag="ps"
                            )
                            for b in range(NB)
                        }
                        jc = slice(js * 128, (js + 1) * 128)
                        for k in range(KA, NJ):
                            for b in range(NB):
                                nc.tensor.matmul(
                                    pacc2[b][:], Wt[:, k, jc], rhs[:, k, b],
                                    start=(k == KA), stop=(k == NJ - 1),
                                )
                        for b in range(NB):
                            nc.vector.tensor_tensor(
                                out_h[:, js, b], pacc2[b][:], parts[js][b][:],
                                op=ALU.add,
                            )
                            so = js * 24 + b * 6
                            nc.vector.bn_stats(
                                t_stats[:, so: so + 6],
                                out_h[:, js, b].bitcast(f32),
                            )
                        aggr(li, js)
                    for j in range(1 + len(RUNWAY_SPLIT), NJ):
                        accs = {
                            b: pspool.tile(
                                [128, 512], f32, name=f"ps_l{li}j{j}_b{b}", tag="ps"
                            )
                            for b in range(NB)
                        }
                        jc = slice(j * 128, (j + 1) * 128)
                        mm_block(accs, Wt, jc, rhs, list(range(NJ)), NJ)
                        drain_stats(li, j, accs, out_h, list(range(NB)),
                                    stats_first=(j >= 5))
                        aggr(li, j)
                        if j == 4:
                            if after_ara is not None:
                                after_ara()
                            collective_group(li, 0, 5, "a")
                st_apply_group(li, 0, 5, "a", out_h)
                collective_group(li, 5, NJ, "b")
                st_apply_group(li, 5, NJ, "b", out_h)

            hidden_layer(2, W2t, h1, h2, after_ara=load_w3)
            hidden_layer(3, W3t, h2, h3)

            # ---------- head: 10-wide binarized linear + sigmoid ----------
            with nc.named_scope("L4"):
                acc4 = {
                    b: pspool.tile([D_OUT, 512], f32, name=f"ps_l4_b{b}", tag="ps")
                    for b in range(NB)
                }
                for k in range(NJ):
                    for b in range(NB):
                        nc.tensor.matmul(
                            acc4[b][:], W4t[:, k], h3[:, k, b],
                            start=(k == 0), stop=(k == NJ - 1),
                        )
                for b in range(NB):
                    osb = outpool.tile([D_OUT, 512], f32, name=f"osb{b}", tag="osb")
                    nc.scalar.activation(osb[:], acc4[b][:], AF.Sigmoid)
                    nc.sync.dma_start(outT[:, b * 512: (b + 1) * 512], osb[:])

    nc.compile()
    return nc


_NC = None
_LAST_RESULTS = None


def _get_nc():
    global _NC
    if _NC is None:
        nc = bacc.Bacc(
            "TRN2", target_bir_lowering=False, debug=False, num_devices=N_CORES
        )
        build(nc)
        _NC = nc
    return _NC


def _axon_reset():
    try:
        import ctypes

        lib = ctypes.CDLL("/opt/axon/libaxon_pjrt.so")
        if hasattr(lib, "axon_reset"):
            lib.axon_reset.restype = ctypes.c_int64
            lib.axon_reset()
    except Exception:
        pass


def kernel(**inputs):
    x = np.ascontiguousarray(inputs["x"], dtype=np.float32)
    w_sign = {}
    for n in ("w1", "w2", "w3", "w4"):
        w = np.asarray(inputs[n], dtype=np.float32)
        w_sign[n] = np.where(w >= 0.0, np.float32(1.0), np.float32(-1.0))
    gb = np.stack(
        [
            np.asarray(inputs[n], dtype=np.float32)
            for n in ("g1", "b1", "g2", "b2", "g3", "b3")
        ]
    )  # [6, 1024]

    def pack_w(ws, kt):
        # [out_feat, in_feat] sign -> [128, kt, out_feat] f32 (lhsT layout)
        kpad = kt * 128
        wt = np.zeros((kpad, ws.shape[0]), np.float32)
        wt[: ws.shape[1]] = ws.T
        return np.ascontiguousarray(
            wt.reshape(kt, 128, ws.shape[0]).transpose(1, 0, 2)
        )

    w1p = pack_w(w_sign["w1"], KT1)
    w2p = pack_w(w_sign["w2"], NJ)
    w3p = pack_w(w_sign["w3"], NJ)
    w4p = pack_w(w_sign["w4"], NJ)
    gbp = np.ascontiguousarray(gb.reshape(6, 8, 128).transpose(2, 0, 1))  # [128,6,8]

    nc = _get_nc()
    in_maps = []
    for c in range(N_CORES):
        xs = np.zeros((D_IN_PAD, BPC), np.float32)
        xs[:D_IN] = x[c * BPC: (c + 1) * BPC].T
        xs = np.ascontiguousarray(
            xs.reshape(KT1, 128, NB, 512).transpose(0, 2, 1, 3)
        )
        in_maps.append(
            {"xT": xs, "w1s": w1p, "w2s": w2p, "w3s": w3p, "w4s": w4p, "gbp": gbp}
        )

    last_err = None
    for _attempt in range(3):
        try:
            res = run_bass_kernel_spmd(nc, in_maps, core_ids=list(range(N_CORES)))
            break
        except Exception as e:  # transient NRT_EXEC_UNIT_UNRECOVERABLE etc.
            last_err = e
            _axon_reset()
    else:
        raise last_err
    global _LAST_RESULTS
    _LAST_RESULTS = res
    out = np.empty((B_TOT, D_OUT), dtype=np.float32)
    for c in range(N_CORES):
        out[c * BPC: (c + 1) * BPC] = res.results[c]["outT"].T
    return out
